# revision 2
# baseline (speedup 1.0000x reference)
"""Distributed causal self-attention kernel for one TRN2 chip (8 NeuronCores).

Problem: y = CausalSelfAttention(x) with B=2, T=2048, C=1024, 16 heads x 64.

Sharding (per core c = b*4 + hg;  b = batch, hg = head-group of 4 heads):
  - Q/K/V projections: column-sharded per head group (each core computes its
    4 heads' Q,K,V from the full x of its batch).
  - Attention: fully local (4 heads per core), flash-style, scores kept
    transposed (s^T[k, q]) so no on-chip transposes are needed.
  - Row-sums for softmax ride the AV matmul as a 65th "ones" column of V.
  - y^T shards are AllGathered within each batch group of 4 cores (two
    gathers, one per head-pair, so comm overlaps the second pair's compute).
  - o_proj: each core computes its own 256 output columns from the full
    gathered y^T -> output shards are disjoint; the host just concatenates.

All matmuls run in bf16 (fp32 accumulation in PSUM); inputs are converted to
bf16 on the host. QK^T matmuls (contraction dim 64) are packed two-per-PE
via tile_position row tiling.
"""
import sys
sys.path.insert(0, '/opt/trn_rl_repo')
import numpy as np
import ml_dtypes

B, T, C = 2, 2048, 1024
NH, HD = 16, 64
N_CORES = 8
GROUPS = [[0, 1, 2, 3], [4, 5, 6, 7]]
HPC = NH // 4            # heads per core = 4
SH = HPC * HD            # per-core projection width = 256
NCB = C // 128           # contraction blocks = 8
QT = 512                 # query tile
BF16 = ml_dtypes.bfloat16

_CACHE = {}


def _build(t_len):
    import concourse.bass as bass
    import concourse.bacc as bacc
    import concourse.tile as tile
    import concourse.mybir as mybir
    dt = mybir.dt
    f32, bf16 = dt.float32, dt.bfloat16

    nqt = t_len // QT        # query tiles
    ntc = t_len // 128       # t chunks of 128
    VW = HPC * 65            # vhat row width = 260

    nc = bacc.Bacc("TRN2", target_bir_lowering=False, debug=False,
                   num_devices=N_CORES)
    xT = nc.dram_tensor("xT", [C, t_len], bf16, kind="ExternalInput")
    wq = nc.dram_tensor("wqT", [C, SH], bf16, kind="ExternalInput")
    wk = nc.dram_tensor("wkT", [C, SH], bf16, kind="ExternalInput")
    wv = nc.dram_tensor("wvT", [C, SH], bf16, kind="ExternalInput")
    wo = nc.dram_tensor("woT", [C, SH], bf16, kind="ExternalInput")
    masks = nc.dram_tensor("masks", [128, 4096], bf16, kind="ExternalInput")
    out = nc.dram_tensor("out", [SH, t_len], f32, kind="ExternalOutput")

    with tile.TileContext(nc) as tc:
        with tc.tile_pool(name="big", bufs=1) as big, \
             tc.tile_pool(name="epool", bufs=4) as epool, \
             tc.tile_pool(name="small", bufs=3) as small, \
             tc.tile_pool(name="ygp", bufs=8) as ygp, \
             tc.tile_pool(name="stp", bufs=3) as stp, \
             tc.tile_pool(name="ps", bufs=4, space="PSUM") as psp, \
             tc.tile_pool(name="dram", bufs=1, space="DRAM") as dram:

            # ---- resident SBUF tensors ----
            xt = big.tile([128, NCB * t_len], bf16)       # x^T, c-blocked
            wq_sb = big.tile([128, NCB * SH], bf16)
            wk_sb = big.tile([128, NCB * SH], bf16)
            wv_sb = big.tile([128, NCB * SH], bf16)
            wo_sb = big.tile([128, NCB * SH], bf16)
            mask_sb = big.tile([128, 4096], bf16)
            qt_sb = big.tile([128, 2 * t_len], bf16)      # Q^T, pair-blocked
            kt_sb = big.tile([128, 2 * t_len], bf16)
            vhat_sb = big.tile([128, ntc * VW], bf16)     # [V_h | 1] per head

            for k in range(NCB):
                nc.sync.dma_start(xt[:, k * t_len:(k + 1) * t_len],
                                  xT[k * 128:(k + 1) * 128, :])
            for w_sb, w_in in ((wq_sb, wq), (wk_sb, wk), (wv_sb, wv), (wo_sb, wo)):
                for k in range(NCB):
                    nc.sync.dma_start(w_sb[:, k * SH:(k + 1) * SH],
                                      w_in[k * 128:(k + 1) * 128, :])
            nc.sync.dma_start(mask_sb[:], masks[:])
            nc.gpsimd.memset(vhat_sb[:], 1.0)

            # ---- DRAM bounce buffers for the AllGathers (one per head pair) ----
            agin = [dram.tile([128, t_len], bf16, name=f"agin{p}") for p in (0, 1)]
            agout = [dram.tile([512, t_len], bf16, name=f"agout{p}") for p in (0, 1)]

            def qk_proj(pair, w_sb, dst_sb):
                """Q^T/K^T for one head pair: dst rows = head dims (2x64)."""
                for n in range(nqt):
                    ps = psp.tile([128, 512], f32, name="ps")
                    for k in range(NCB):
                        nc.tensor.matmul(
                            ps[:],
                            lhsT=w_sb[:, k * SH + pair * 128: k * SH + (pair + 1) * 128],
                            rhs=xt[:, k * t_len + n * QT: k * t_len + n * QT + QT],
                            start=(k == 0), stop=(k == NCB - 1))
                    nc.vector.tensor_copy(
                        dst_sb[:, pair * t_len + n * QT: pair * t_len + n * QT + QT],
                        ps[:])

            def v_proj():
                """V in [t, o] layout, written per head into vhat (col 65 stays 1)."""
                for tch in range(ntc):
                    ps = psp.tile([128, SH], f32, name="ps")
                    for k in range(NCB):
                        nc.tensor.matmul(
                            ps[:],
                            lhsT=xt[:, k * t_len + tch * 128: k * t_len + (tch + 1) * 128],
                            rhs=wv_sb[:, k * SH:(k + 1) * SH],
                            start=(k == 0), stop=(k == NCB - 1))
                    for h in range(HPC):
                        nc.vector.tensor_copy(
                            vhat_sb[:, tch * VW + h * 65: tch * VW + h * 65 + 64],
                            ps[:, h * 64:(h + 1) * 64])

            def attention(pair):
                for qi in range(nqt):
                    q0 = qi * QT
                    nkb = (q0 + QT) // 128
                    aug = psp.tile([128, 1024], f32, name="ps")
                    for kb in range(nkb):
                        qk = psp.tile([128, 1024], f32, name="ps")
                        for h01 in (0, 1):
                            nc.tensor.matmul(
                                qk[:, h01 * 512:(h01 + 1) * 512],
                                lhsT=kt_sb[h01 * 64:(h01 + 1) * 64,
                                           pair * t_len + kb * 128: pair * t_len + (kb + 1) * 128],
                                rhs=qt_sb[h01 * 64:(h01 + 1) * 64,
                                          pair * t_len + q0: pair * t_len + q0 + QT],
                                start=True, stop=True,
                                tile_position=(h01 * 64, 0))
                        e = epool.tile([128, 1024], bf16, name="e")
                        nc.scalar.activation(e[:], qk[:],
                                             mybir.ActivationFunctionType.Exp,
                                             scale=1.0 / np.sqrt(HD))
                        j = kb - q0 // 128
                        if j >= 0:
                            nc.vector.tensor_mul(e[:], e[:],
                                                 mask_sb[:, j * 1024:(j + 1) * 1024])
                        for h01 in (0, 1):
                            h = pair * 2 + h01
                            nc.tensor.matmul(
                                aug[0:65, h01 * 512:(h01 + 1) * 512],
                                lhsT=vhat_sb[:, kb * VW + h * 65: kb * VW + (h + 1) * 65],
                                rhs=e[:, h01 * 512:(h01 + 1) * 512],
                                start=(kb == 0), stop=(kb == nkb - 1))
                    # normalize: y^T_h = aug[0:64] / aug[64]
                    recip = small.tile([1, 1024], bf16, name="recip")
                    with nc.allow_low_precision(reason="softmax denom in bf16 is within tolerance"):
                        nc.vector.reciprocal(recip[:], aug[64:65, 0:1024])
                    bc = small.tile([64, 1024], bf16, name="bc")
                    nc.gpsimd.partition_broadcast(bc[:], recip[:])
                    yt = small.tile([64, 1024], bf16, name="yt")
                    nc.vector.tensor_mul(yt[:], aug[0:64, 0:1024], bc[:])
                    for h01 in (0, 1):
                        nc.sync.dma_start(
                            agin[pair][h01 * 64:(h01 + 1) * 64, q0:q0 + QT],
                            yt[:, h01 * 512:(h01 + 1) * 512])

            # ---- schedule: pair0 projections -> attention p0 (ACT-bound)
            #      overlapped with pair1 projections -> attention p1 ----
            qk_proj(0, wq_sb, qt_sb)
            qk_proj(0, wk_sb, kt_sb)
            v_proj()
            attention(0)
            qk_proj(1, wq_sb, qt_sb)
            qk_proj(1, wk_sb, kt_sb)
            attention(1)

            for p in (0, 1):
                nc.gpsimd.collective_compute(
                    "AllGather", mybir.AluOpType.bypass,
                    replica_groups=GROUPS,
                    ins=[agin[p].opt()], outs=[agout[p].opt()])

            # ---- gathered y^T -> SBUF ----
            yg = []
            for p in (0, 1):
                for r in range(4):
                    t = ygp.tile([128, t_len], bf16, name="yg")
                    nc.sync.dma_start(t[:], agout[p][r * 128:(r + 1) * 128, :])
                    yg.append((2 * r + p, t))   # global c-block index

            # ---- o_proj: out^T[o, t] = sum_c woT[c, o] * yg[c, t] ----
            for m in range(2):
                for n in range(nqt):
                    ps = psp.tile([128, 512], f32, name="ps")
                    for i, (cb, t) in enumerate(yg):
                        nc.tensor.matmul(
                            ps[:],
                            lhsT=wo_sb[:, cb * SH + m * 128: cb * SH + (m + 1) * 128],
                            rhs=t[:, n * QT: n * QT + QT],
                            start=(i == 0), stop=(i == len(yg) - 1))
                    st = stp.tile([128, 512], f32, name="st")
                    nc.vector.tensor_copy(st[:], ps[:])
                    nc.sync.dma_start(
                        out[m * 128:(m + 1) * 128, n * QT: n * QT + QT], st[:])

    nc.compile()
    return nc


def _masks_np():
    """Diagonal causal masks: mask[j][ki, qi] = qi >= j*128 + ki, duplicated
    along the free axis for the two packed heads."""
    ki = np.arange(128)[:, None]
    qi = np.arange(512)[None, :]
    ms = []
    for j in range(4):
        m = (qi >= j * 128 + ki).astype(BF16)
        ms.append(np.concatenate([m, m], axis=1))
    return np.concatenate(ms, axis=1).astype(BF16)  # [128, 4096]


def _prep_inputs(x, Wq, Wk, Wv, Wo, t_len):
    masks = _masks_np()
    in_maps = []
    for c in range(N_CORES):
        b, hg = divmod(c, 4)
        sl = slice(hg * SH, (hg + 1) * SH)
        in_maps.append({
            "xT": np.ascontiguousarray(x[b].T).astype(BF16),
            "wqT": np.ascontiguousarray(Wq[sl, :].T).astype(BF16),
            "wkT": np.ascontiguousarray(Wk[sl, :].T).astype(BF16),
            "wvT": np.ascontiguousarray(Wv[sl, :].T).astype(BF16),
            "woT": np.ascontiguousarray(Wo[sl, :].T).astype(BF16),
            "masks": masks,
        })
    return in_maps


def _assemble(results, t_len):
    out = np.empty((B, t_len, C), dtype=np.float32)
    for c in range(N_CORES):
        b, hg = divmod(c, 4)
        out[b, :, hg * SH:(hg + 1) * SH] = results[c]["out"].T
    return out


def get_nc(t_len=T):
    if t_len not in _CACHE:
        _CACHE[t_len] = _build(t_len)
    return _CACHE[t_len]


def kernel(x, Wq, Wk, Wv, Wo):
    from concourse import bass_utils
    x = np.asarray(x, dtype=np.float32)
    nc = get_nc(T)
    in_maps = _prep_inputs(x, np.asarray(Wq), np.asarray(Wk), np.asarray(Wv),
                           np.asarray(Wo), T)
    res = bass_utils.run_bass_kernel_spmd(nc, in_maps, core_ids=list(range(N_CORES)))
    return _assemble(res.results, T)


# revision 19
# speedup vs baseline: 1.0200x; 1.0200x over previous
"""Distributed causal self-attention kernel for one TRN2 chip (8 NeuronCores).

Problem: y = CausalSelfAttention(x) with B=2, T=2048, C=1024, 16 heads x 64.

Sharding (per core c = b*4 + hg;  b = batch, hg = head-group of 4 heads):
  - Q/K/V projections: column-sharded per head group (each core computes its
    4 heads' Q,K,V from the full x of its batch).
  - Attention: fully local (4 heads per core), flash-style, scores kept
    transposed (s^T[k, q]) so no on-chip transposes are needed.
  - Row-sums for softmax ride the AV matmul as a 65th "ones" column of V.
  - y^T shards are AllGathered within each batch group of 4 cores (two
    gathers, one per head-pair, so comm overlaps the second pair's compute).
  - o_proj: each core computes its own 256 output columns from the full
    gathered y^T -> output shards are disjoint; the host just concatenates.

All matmuls run in bf16 (fp32 accumulation in PSUM); inputs are converted to
bf16 on the host. QK^T matmuls (contraction dim 64) are packed two-per-PE
via tile_position row tiling.
"""
import sys
sys.path.insert(0, '/opt/trn_rl_repo')
import numpy as np
import ml_dtypes

B, T, C = 2, 2048, 1024
NH, HD = 16, 64
N_CORES = 8
GROUPS = [[0, 1, 2, 3], [4, 5, 6, 7]]
HPC = NH // 4            # heads per core = 4
SH = HPC * HD            # per-core projection width = 256
NCB = C // 128           # contraction blocks = 8
QT = 512                 # query tile
BF16 = ml_dtypes.bfloat16

_CACHE = {}


def _build(t_len):
    import concourse.bass as bass
    import concourse.bacc as bacc
    import concourse.tile as tile
    import concourse.mybir as mybir
    dt = mybir.dt
    f32, bf16 = dt.float32, dt.bfloat16

    nqt = t_len // QT        # query tiles
    ntc = t_len // 128       # t chunks of 128
    VW = HPC * 65            # vhat row width = 260

    nc = bacc.Bacc("TRN2", target_bir_lowering=False, debug=False,
                   num_devices=N_CORES)
    # inputs arrive pre-blocked on the host: [(cblk p) ...] -> [p, cblk*...]
    xT = nc.dram_tensor("xT", [128, NCB * t_len], bf16, kind="ExternalInput")
    wq = nc.dram_tensor("wqT", [128, NCB * SH], bf16, kind="ExternalInput")
    wk = nc.dram_tensor("wkT", [128, NCB * SH], bf16, kind="ExternalInput")
    wv = nc.dram_tensor("wvT", [128, NCB * SH], bf16, kind="ExternalInput")
    wo = nc.dram_tensor("woT", [128, NCB * SH], bf16, kind="ExternalInput")
    masks = nc.dram_tensor("masks", [128, 1024], bf16, kind="ExternalInput")
    out = nc.dram_tensor("out", [SH, t_len], f32, kind="ExternalOutput")

    with tile.TileContext(nc) as tc:
        with tc.tile_pool(name="big", bufs=1) as big, \
             tc.tile_pool(name="epool", bufs=4) as epool, \
             tc.tile_pool(name="small", bufs=3) as small, \
             tc.tile_pool(name="ygp", bufs=8) as ygp, \
             tc.tile_pool(name="stp", bufs=3) as stp, \
             tc.tile_pool(name="ps", bufs=4, space="PSUM") as psp, \
             tc.tile_pool(name="dram", bufs=1, space="DRAM") as dram:

            # ---- resident SBUF tensors ----
            xt = big.tile([128, NCB * t_len], bf16)       # x^T, c-blocked
            wq_sb = big.tile([128, NCB * SH], bf16)
            wk_sb = big.tile([128, NCB * SH], bf16)
            wv_sb = big.tile([128, NCB * SH], bf16)
            wo_sb = big.tile([128, NCB * SH], bf16)
            mask_sb = big.tile([128, 1024], bf16)
            qt_sb = big.tile([128, 2 * t_len], bf16)      # Q^T, pair-blocked
            kt_sb = big.tile([128, 2 * t_len], bf16)
            vhat_sb = big.tile([128, ntc * VW], bf16)     # [V_h | 1] per head

            for k in range(NCB):
                nc.sync.dma_start(xt[:, k * t_len:(k + 1) * t_len],
                                  xT[:, k * t_len:(k + 1) * t_len])
            for w_sb, w_in in ((wq_sb, wq), (wk_sb, wk), (wv_sb, wv), (wo_sb, wo)):
                for k in range(NCB):
                    nc.sync.dma_start(w_sb[:, k * SH:(k + 1) * SH],
                                      w_in[:, k * SH:(k + 1) * SH])
            nc.sync.dma_start(mask_sb[:], masks[:])
            nc.gpsimd.memset(vhat_sb[:], 1.0)

            # ---- DRAM bounce buffers for the AllGathers (pair x t-half) ----
            n_th = max(1, t_len // 1024)
            th_len = t_len // n_th
            agin = [[dram.tile([128, th_len], bf16, name=f"agin{p}{th}")
                     for th in range(n_th)] for p in (0, 1)]
            agout = [[dram.tile([512, th_len], bf16, name=f"agout{p}{th}")
                      for th in range(n_th)] for p in (0, 1)]

            def qk_proj(pair, w_sb, dst_sb):
                """Q^T/K^T for one head pair: dst rows = head dims (2x64)."""
                for n in range(nqt):
                    ps = psp.tile([128, 512], f32, name="ps")
                    for k in range(NCB):
                        nc.tensor.matmul(
                            ps[:],
                            lhsT=w_sb[:, k * SH + pair * 128: k * SH + (pair + 1) * 128],
                            rhs=xt[:, k * t_len + n * QT: k * t_len + n * QT + QT],
                            start=(k == 0), stop=(k == NCB - 1))
                    nc.vector.tensor_copy(
                        dst_sb[:, pair * t_len + n * QT: pair * t_len + n * QT + QT],
                        ps[:])

            def v_proj():
                """V in [t, o] layout, written per head into vhat (col 65 stays 1)."""
                for tch in range(ntc):
                    ps = psp.tile([128, SH], f32, name="ps")
                    for k in range(NCB):
                        nc.tensor.matmul(
                            ps[:],
                            lhsT=xt[:, k * t_len + tch * 128: k * t_len + (tch + 1) * 128],
                            rhs=wv_sb[:, k * SH:(k + 1) * SH],
                            start=(k == 0), stop=(k == NCB - 1))
                    for h in range(HPC):
                        nc.vector.tensor_copy(
                            vhat_sb[:, tch * VW + h * 65: tch * VW + h * 65 + 64],
                            ps[:, h * 64:(h + 1) * 64])

            def attention(pair):
                def qk_mm(dst, kb, qa, w, h01):
                    """s^T block matmul: k-block kb vs q cols [qa, qa+w)."""
                    nc.tensor.matmul(
                        dst,
                        lhsT=kt_sb[h01 * 64:(h01 + 1) * 64,
                                   pair * t_len + kb * 128: pair * t_len + (kb + 1) * 128],
                        rhs=qt_sb[h01 * 64:(h01 + 1) * 64,
                                  pair * t_len + qa: pair * t_len + qa + w],
                        start=True, stop=True,
                        tile_position=(h01 * 64, 0))

                def av_mm(aug, e_slice, kb, h01, ca, w, start, stop):
                    h = pair * 2 + h01
                    return nc.tensor.matmul(
                        aug[0:65, h01 * 512 + ca: h01 * 512 + ca + w],
                        lhsT=vhat_sb[:, kb * VW + h * 65: kb * VW + (h + 1) * 65],
                        rhs=e_slice,
                        start=start, stop=stop,
                        skip_group_check=True)

                for qi in range(nqt):
                    q0 = qi * QT
                    nfull = q0 // 128          # k-blocks fully valid for all 512 q
                    aug = psp.tile([128, 1024], f32, name="ps")
                    for kb in range(nfull):
                        qk = psp.tile([128, 1024], f32, name="ps")
                        for h01 in (0, 1):
                            qk_mm(qk[:, h01 * 512:(h01 + 1) * 512], kb, q0, 512, h01)
                        e = epool.tile([128, 1024], bf16, name="e")
                        nc.scalar.activation(e[:], qk[:],
                                             mybir.ActivationFunctionType.Exp,
                                             scale=1.0 / np.sqrt(HD))
                        for h01 in (0, 1):
                            av_mm(aug, e[:, h01 * 512:(h01 + 1) * 512], kb, h01,
                                  0, 512, start=(kb == 0), stop=False)
                    # mid supertile: blocks nfull, nfull+1 are fully valid for
                    # the upper q-half [q0+256, q0+512). Packed (i, h01) x 256.
                    mid = psp.tile([128, 1024], f32, name="ps")
                    for i in (0, 1):
                        for h01 in (0, 1):
                            qk_mm(mid[:, (h01 * 2 + i) * 256:(h01 * 2 + i + 1) * 256],
                                  nfull + i, q0 + 256, 256, h01)
                    em = epool.tile([128, 1024], bf16, name="e")
                    nc.scalar.activation(em[:], mid[:],
                                         mybir.ActivationFunctionType.Exp,
                                         scale=1.0 / np.sqrt(HD))
                    for i in (0, 1):
                        for h01 in (0, 1):
                            av_mm(aug, em[:, (h01 * 2 + i) * 256:(h01 * 2 + i + 1) * 256],
                                  nfull + i, h01, 256, 256,
                                  start=(nfull == 0 and i == 0), stop=False)
                    # Two diagonal bands: band u covers q-half [q0+u*256, +256)
                    # against k-blocks nfull+2u, nfull+2u+1 with the causal mask.
                    # PSUM accumulation-group discipline: a start=True matmul
                    # into a bank clobbers any OPEN group in that bank, so band1
                    # (which closes the [256,512) group opened by full/mid) must
                    # fully precede band0's start when nfull==0. Band order
                    # (1, 0) plus an explicit dep enforces this.
                    band_last_av = None
                    band0_first_av = None
                    for u in (1, 0):
                        bd = psp.tile([128, 1024], f32, name="ps")
                        for i in (0, 1):
                            for h01 in (0, 1):
                                qk_mm(bd[:, (h01 * 2 + i) * 256:(h01 * 2 + i + 1) * 256],
                                      nfull + 2 * u + i, q0 + u * 256, 256, h01)
                        eb = epool.tile([128, 1024], bf16, name="e")
                        nc.scalar.activation(eb[:], bd[:],
                                             mybir.ActivationFunctionType.Exp,
                                             scale=1.0 / np.sqrt(HD))
                        nc.vector.tensor_mul(eb[:], eb[:], mask_sb[:])
                        for i in (0, 1):
                            for h01 in (0, 1):
                                av = av_mm(aug, eb[:, (h01 * 2 + i) * 256:(h01 * 2 + i + 1) * 256],
                                           nfull + 2 * u + i, h01, u * 256, 256,
                                           start=(nfull == 0 and u == 0 and i == 0),
                                           stop=(i == 1))
                                if u == 1:
                                    band_last_av = av
                                elif band0_first_av is None:
                                    band0_first_av = av
                    if nfull == 0 and band_last_av is not None:
                        tile.add_dep_helper(band0_first_av.ins, band_last_av.ins,
                                            reason="bank group: band0 start after band1 closes")
                    # normalize: y^T_h = aug[0:64] / aug[64]
                    recip = small.tile([1, 1024], bf16, name="recip")
                    with nc.allow_low_precision(reason="softmax denom in bf16 is within tolerance"):
                        nc.vector.reciprocal(recip[:], aug[64:65, 0:1024])
                    bc = small.tile([64, 1024], bf16, name="bc")
                    nc.gpsimd.partition_broadcast(bc[:], recip[:])
                    yt = small.tile([64, 1024], bf16, name="yt")
                    nc.vector.tensor_mul(yt[:], aug[0:64, 0:1024], bc[:])
                    th, tq = divmod(q0, th_len)
                    nc.sync.dma_start(
                        agin[pair][th].rearrange("(h d) t -> d h t", h=2)[:, :, tq:tq + QT],
                        yt.rearrange("d (h t) -> d h t", h=2))

            # ---- schedule: pair0 projections -> attention p0 (ACT-bound)
            #      overlapped with pair1 projections -> attention p1 ----
            qk_proj(0, wq_sb, qt_sb)
            qk_proj(0, wk_sb, kt_sb)
            v_proj()
            attention(0)
            qk_proj(1, wq_sb, qt_sb)
            qk_proj(1, wk_sb, kt_sb)
            attention(1)

            for th in range(n_th):
                for p in (0, 1):
                    nc.gpsimd.collective_compute(
                        "AllGather", mybir.AluOpType.bypass,
                        replica_groups=GROUPS,
                        ins=[agin[p][th].opt()], outs=[agout[p][th].opt()])

            # ---- gathered y^T -> SBUF ----
            yg = {}  # (global c-block, t-half) -> sbuf tile
            for th in range(n_th):
                for p in (0, 1):
                    for r in range(4):
                        t = ygp.tile([128, th_len], bf16, name="yg")
                        nc.sync.dma_start(t[:], agout[p][th][r * 128:(r + 1) * 128, :])
                        yg[(2 * r + p, th)] = t

            # ---- o_proj: out^T[o, t] = sum_c woT[c, o] * yg[c, t] ----
            nth = th_len // QT
            for n in range(nqt):
                th, nn = divmod(n, nth)
                for m in range(2):
                    ps = psp.tile([128, 512], f32, name="ps")
                    for cb in range(NCB):
                        nc.tensor.matmul(
                            ps[:],
                            lhsT=wo_sb[:, cb * SH + m * 128: cb * SH + (m + 1) * 128],
                            rhs=yg[(cb, th)][:, nn * QT: nn * QT + QT],
                            start=(cb == 0), stop=(cb == NCB - 1))
                    st = stp.tile([128, 512], f32, name="st")
                    nc.vector.tensor_copy(st[:], ps[:])
                    nc.sync.dma_start(
                        out[m * 128:(m + 1) * 128, n * QT: n * QT + QT], st[:])

    nc.compile()
    return nc


def _masks_np():
    """Diagonal causal mask: [ki, qi] = qi >= ki, duplicated along the free
    axis for the two packed heads."""
    ki = np.arange(128)[:, None]
    qi = np.arange(128)[None, :]
    tri = (qi >= ki).astype(np.float32)
    ones = np.ones((128, 128), np.float32)
    zeros = np.zeros((128, 128), np.float32)
    lo = np.concatenate([tri, ones], axis=1)    # lower k-block of a band
    hi = np.concatenate([zeros, tri], axis=1)   # upper k-block of a band
    return np.concatenate([lo, hi, lo, hi], axis=1).astype(BF16)  # [128, 1024]


def _block(a, w):
    """[C, w] -> [128, NCB*w] partition-blocked bf16."""
    return np.ascontiguousarray(
        a.reshape(NCB, 128, w).transpose(1, 0, 2).reshape(128, NCB * w)).astype(BF16)


def _prep_inputs(x, Wq, Wk, Wv, Wo, t_len):
    masks = _masks_np()
    in_maps = []
    for c in range(N_CORES):
        b, hg = divmod(c, 4)
        sl = slice(hg * SH, (hg + 1) * SH)
        in_maps.append({
            "xT": _block(x[b].T, t_len),
            "wqT": _block(Wq[sl, :].T, SH),
            "wkT": _block(Wk[sl, :].T, SH),
            "wvT": _block(Wv[sl, :].T, SH),
            "woT": _block(Wo[sl, :].T, SH),
            "masks": masks,
        })
    return in_maps


def _assemble(results, t_len):
    out = np.empty((B, t_len, C), dtype=np.float32)
    for c in range(N_CORES):
        b, hg = divmod(c, 4)
        out[b, :, hg * SH:(hg + 1) * SH] = results[c]["out"].T
    return out


def get_nc(t_len=T):
    if t_len not in _CACHE:
        _CACHE[t_len] = _build(t_len)
    return _CACHE[t_len]


def kernel(x, Wq, Wk, Wv, Wo):
    from concourse import bass_utils
    x = np.asarray(x, dtype=np.float32)
    nc = get_nc(T)
    in_maps = _prep_inputs(x, np.asarray(Wq), np.asarray(Wk), np.asarray(Wv),
                           np.asarray(Wo), T)
    res = bass_utils.run_bass_kernel_spmd(nc, in_maps, core_ids=list(range(N_CORES)))
    return _assemble(res.results, T)


# revision 20
# speedup vs baseline: 1.0213x; 1.0013x over previous
"""Distributed causal self-attention kernel for one TRN2 chip (8 NeuronCores).

Problem: y = CausalSelfAttention(x) with B=2, T=2048, C=1024, 16 heads x 64.

Sharding (per core c = b*4 + hg;  b = batch, hg = head-group of 4 heads):
  - Q/K/V projections: column-sharded per head group (each core computes its
    4 heads' Q,K,V from the full x of its batch).
  - Attention: fully local (4 heads per core), flash-style, scores kept
    transposed (s^T[k, q]) so no on-chip transposes are needed.
  - Row-sums for softmax ride the AV matmul as a 65th "ones" column of V.
  - y^T shards are AllGathered within each batch group of 4 cores (two
    gathers, one per head-pair, so comm overlaps the second pair's compute).
  - o_proj: each core computes its own 256 output columns from the full
    gathered y^T -> output shards are disjoint; the host just concatenates.

All matmuls run in bf16 (fp32 accumulation in PSUM); inputs are converted to
bf16 on the host. QK^T matmuls (contraction dim 64) are packed two-per-PE
via tile_position row tiling.
"""
import sys
sys.path.insert(0, '/opt/trn_rl_repo')
import numpy as np
import ml_dtypes

B, T, C = 2, 2048, 1024
NH, HD = 16, 64
N_CORES = 8
GROUPS = [[0, 1, 2, 3], [4, 5, 6, 7]]
HPC = NH // 4            # heads per core = 4
SH = HPC * HD            # per-core projection width = 256
NCB = C // 128           # contraction blocks = 8
QT = 512                 # query tile
BF16 = ml_dtypes.bfloat16

_CACHE = {}


def _build(t_len):
    import concourse.bass as bass
    import concourse.bacc as bacc
    import concourse.tile as tile
    import concourse.mybir as mybir
    dt = mybir.dt
    f32, bf16 = dt.float32, dt.bfloat16

    nqt = t_len // QT        # query tiles
    ntc = t_len // 128       # t chunks of 128
    VW = HPC * 65            # vhat row width = 260

    nc = bacc.Bacc("TRN2", target_bir_lowering=False, debug=False,
                   num_devices=N_CORES)
    # inputs arrive pre-blocked on the host: [(cblk p) ...] -> [p, cblk*...]
    xT = nc.dram_tensor("xT", [128, NCB * t_len], bf16, kind="ExternalInput")
    wq = nc.dram_tensor("wqT", [128, NCB * SH], bf16, kind="ExternalInput")
    wk = nc.dram_tensor("wkT", [128, NCB * SH], bf16, kind="ExternalInput")
    wv = nc.dram_tensor("wvT", [128, NCB * SH], bf16, kind="ExternalInput")
    wo = nc.dram_tensor("woT", [128, NCB * SH], bf16, kind="ExternalInput")
    masks = nc.dram_tensor("masks", [128, 1024], bf16, kind="ExternalInput")
    out = nc.dram_tensor("out", [SH, t_len], bf16, kind="ExternalOutput")

    with tile.TileContext(nc) as tc:
        with tc.tile_pool(name="big", bufs=1) as big, \
             tc.tile_pool(name="epool", bufs=4) as epool, \
             tc.tile_pool(name="small", bufs=3) as small, \
             tc.tile_pool(name="ygp", bufs=8) as ygp, \
             tc.tile_pool(name="stp", bufs=3) as stp, \
             tc.tile_pool(name="ps", bufs=4, space="PSUM") as psp, \
             tc.tile_pool(name="dram", bufs=1, space="DRAM") as dram:

            # ---- resident SBUF tensors ----
            xt = big.tile([128, NCB * t_len], bf16)       # x^T, c-blocked
            wq_sb = big.tile([128, NCB * SH], bf16)
            wk_sb = big.tile([128, NCB * SH], bf16)
            wv_sb = big.tile([128, NCB * SH], bf16)
            wo_sb = big.tile([128, NCB * SH], bf16)
            mask_sb = big.tile([128, 1024], bf16)
            qt_sb = big.tile([128, 2 * t_len], bf16)      # Q^T, pair-blocked
            kt_sb = big.tile([128, 2 * t_len], bf16)
            vhat_sb = big.tile([128, ntc * VW], bf16)     # [V_h | 1] per head

            for k in range(NCB):
                nc.sync.dma_start(xt[:, k * t_len:(k + 1) * t_len],
                                  xT[:, k * t_len:(k + 1) * t_len])
            for w_sb, w_in in ((wq_sb, wq), (wk_sb, wk), (wv_sb, wv), (wo_sb, wo)):
                for k in range(NCB):
                    nc.sync.dma_start(w_sb[:, k * SH:(k + 1) * SH],
                                      w_in[:, k * SH:(k + 1) * SH])
            nc.sync.dma_start(mask_sb[:], masks[:])
            nc.gpsimd.memset(vhat_sb[:], 1.0)

            # ---- DRAM bounce buffers for the AllGathers (pair x t-half) ----
            n_th = max(1, t_len // 1024)
            th_len = t_len // n_th
            agin = [[dram.tile([128, th_len], bf16, name=f"agin{p}{th}")
                     for th in range(n_th)] for p in (0, 1)]
            agout = [[dram.tile([512, th_len], bf16, name=f"agout{p}{th}")
                      for th in range(n_th)] for p in (0, 1)]

            def qk_proj(pair, w_sb, dst_sb):
                """Q^T/K^T for one head pair: dst rows = head dims (2x64)."""
                for n in range(nqt):
                    ps = psp.tile([128, 512], f32, name="ps")
                    for k in range(NCB):
                        nc.tensor.matmul(
                            ps[:],
                            lhsT=w_sb[:, k * SH + pair * 128: k * SH + (pair + 1) * 128],
                            rhs=xt[:, k * t_len + n * QT: k * t_len + n * QT + QT],
                            start=(k == 0), stop=(k == NCB - 1))
                    nc.vector.tensor_copy(
                        dst_sb[:, pair * t_len + n * QT: pair * t_len + n * QT + QT],
                        ps[:])

            def v_proj():
                """V in [t, o] layout, written per head into vhat (col 65 stays 1)."""
                for tch in range(ntc):
                    ps = psp.tile([128, SH], f32, name="ps")
                    for k in range(NCB):
                        nc.tensor.matmul(
                            ps[:],
                            lhsT=xt[:, k * t_len + tch * 128: k * t_len + (tch + 1) * 128],
                            rhs=wv_sb[:, k * SH:(k + 1) * SH],
                            start=(k == 0), stop=(k == NCB - 1))
                    for h in range(HPC):
                        nc.vector.tensor_copy(
                            vhat_sb[:, tch * VW + h * 65: tch * VW + h * 65 + 64],
                            ps[:, h * 64:(h + 1) * 64])

            def attention(pair):
                def qk_mm(dst, kb, qa, w, h01):
                    """s^T block matmul: k-block kb vs q cols [qa, qa+w)."""
                    nc.tensor.matmul(
                        dst,
                        lhsT=kt_sb[h01 * 64:(h01 + 1) * 64,
                                   pair * t_len + kb * 128: pair * t_len + (kb + 1) * 128],
                        rhs=qt_sb[h01 * 64:(h01 + 1) * 64,
                                  pair * t_len + qa: pair * t_len + qa + w],
                        start=True, stop=True,
                        tile_position=(h01 * 64, 0))

                def av_mm(aug, e_slice, kb, h01, ca, w, start, stop):
                    h = pair * 2 + h01
                    return nc.tensor.matmul(
                        aug[0:65, h01 * 512 + ca: h01 * 512 + ca + w],
                        lhsT=vhat_sb[:, kb * VW + h * 65: kb * VW + (h + 1) * 65],
                        rhs=e_slice,
                        start=start, stop=stop,
                        skip_group_check=True)

                for qi in range(nqt):
                    q0 = qi * QT
                    nfull = q0 // 128          # k-blocks fully valid for all 512 q
                    aug = psp.tile([128, 1024], f32, name="ps")
                    for kb in range(nfull):
                        qk = psp.tile([128, 1024], f32, name="ps")
                        for h01 in (0, 1):
                            qk_mm(qk[:, h01 * 512:(h01 + 1) * 512], kb, q0, 512, h01)
                        e = epool.tile([128, 1024], bf16, name="e")
                        nc.scalar.activation(e[:], qk[:],
                                             mybir.ActivationFunctionType.Exp,
                                             scale=1.0 / np.sqrt(HD))
                        for h01 in (0, 1):
                            av_mm(aug, e[:, h01 * 512:(h01 + 1) * 512], kb, h01,
                                  0, 512, start=(kb == 0), stop=False)
                    # mid supertile: blocks nfull, nfull+1 are fully valid for
                    # the upper q-half [q0+256, q0+512). Packed (i, h01) x 256.
                    mid = psp.tile([128, 1024], f32, name="ps")
                    for i in (0, 1):
                        for h01 in (0, 1):
                            qk_mm(mid[:, (h01 * 2 + i) * 256:(h01 * 2 + i + 1) * 256],
                                  nfull + i, q0 + 256, 256, h01)
                    em = epool.tile([128, 1024], bf16, name="e")
                    nc.scalar.activation(em[:], mid[:],
                                         mybir.ActivationFunctionType.Exp,
                                         scale=1.0 / np.sqrt(HD))
                    for i in (0, 1):
                        for h01 in (0, 1):
                            av_mm(aug, em[:, (h01 * 2 + i) * 256:(h01 * 2 + i + 1) * 256],
                                  nfull + i, h01, 256, 256,
                                  start=(nfull == 0 and i == 0), stop=False)
                    # Two diagonal bands: band u covers q-half [q0+u*256, +256)
                    # against k-blocks nfull+2u, nfull+2u+1 with the causal mask.
                    # PSUM accumulation-group discipline: a start=True matmul
                    # into a bank clobbers any OPEN group in that bank, so band1
                    # (which closes the [256,512) group opened by full/mid) must
                    # fully precede band0's start when nfull==0. Band order
                    # (1, 0) plus an explicit dep enforces this.
                    band_last_av = None
                    band0_first_av = None
                    for u in (1, 0):
                        bd = psp.tile([128, 1024], f32, name="ps")
                        for i in (0, 1):
                            for h01 in (0, 1):
                                qk_mm(bd[:, (h01 * 2 + i) * 256:(h01 * 2 + i + 1) * 256],
                                      nfull + 2 * u + i, q0 + u * 256, 256, h01)
                        eb = epool.tile([128, 1024], bf16, name="e")
                        nc.scalar.activation(eb[:], bd[:],
                                             mybir.ActivationFunctionType.Exp,
                                             scale=1.0 / np.sqrt(HD))
                        nc.vector.tensor_mul(eb[:], eb[:], mask_sb[:])
                        for i in (0, 1):
                            for h01 in (0, 1):
                                av = av_mm(aug, eb[:, (h01 * 2 + i) * 256:(h01 * 2 + i + 1) * 256],
                                           nfull + 2 * u + i, h01, u * 256, 256,
                                           start=(nfull == 0 and u == 0 and i == 0),
                                           stop=(i == 1))
                                if u == 1:
                                    band_last_av = av
                                elif band0_first_av is None:
                                    band0_first_av = av
                    if nfull == 0 and band_last_av is not None:
                        tile.add_dep_helper(band0_first_av.ins, band_last_av.ins,
                                            reason="bank group: band0 start after band1 closes")
                    # normalize: y^T_h = aug[0:64] / aug[64]
                    recip = small.tile([1, 1024], bf16, name="recip")
                    with nc.allow_low_precision(reason="softmax denom in bf16 is within tolerance"):
                        nc.vector.reciprocal(recip[:], aug[64:65, 0:1024])
                    bc = small.tile([64, 1024], bf16, name="bc")
                    nc.gpsimd.partition_broadcast(bc[:], recip[:])
                    yt = small.tile([64, 1024], bf16, name="yt")
                    nc.vector.tensor_mul(yt[:], aug[0:64, 0:1024], bc[:])
                    th, tq = divmod(q0, th_len)
                    nc.sync.dma_start(
                        agin[pair][th].rearrange("(h d) t -> d h t", h=2)[:, :, tq:tq + QT],
                        yt.rearrange("d (h t) -> d h t", h=2))

            # ---- schedule: pair0 projections -> attention p0 (ACT-bound)
            #      overlapped with pair1 projections -> attention p1 ----
            qk_proj(0, wq_sb, qt_sb)
            qk_proj(0, wk_sb, kt_sb)
            v_proj()
            attention(0)
            qk_proj(1, wq_sb, qt_sb)
            qk_proj(1, wk_sb, kt_sb)
            attention(1)

            for th in range(n_th):
                for p in (0, 1):
                    nc.gpsimd.collective_compute(
                        "AllGather", mybir.AluOpType.bypass,
                        replica_groups=GROUPS,
                        ins=[agin[p][th].opt()], outs=[agout[p][th].opt()])

            # ---- gathered y^T -> SBUF ----
            yg = {}  # (global c-block, t-half) -> sbuf tile
            for th in range(n_th):
                for p in (0, 1):
                    for r in range(4):
                        t = ygp.tile([128, th_len], bf16, name="yg")
                        nc.sync.dma_start(t[:], agout[p][th][r * 128:(r + 1) * 128, :])
                        yg[(2 * r + p, th)] = t

            # ---- o_proj: out^T[o, t] = sum_c woT[c, o] * yg[c, t] ----
            nth = th_len // QT
            for n in range(nqt):
                th, nn = divmod(n, nth)
                for m in range(2):
                    ps = psp.tile([128, 512], f32, name="ps")
                    for cb in range(NCB):
                        nc.tensor.matmul(
                            ps[:],
                            lhsT=wo_sb[:, cb * SH + m * 128: cb * SH + (m + 1) * 128],
                            rhs=yg[(cb, th)][:, nn * QT: nn * QT + QT],
                            start=(cb == 0), stop=(cb == NCB - 1))
                    st = stp.tile([128, 512], bf16, name="st")
                    nc.vector.tensor_copy(st[:], ps[:])
                    nc.sync.dma_start(
                        out[m * 128:(m + 1) * 128, n * QT: n * QT + QT], st[:])

    nc.compile()
    return nc


def _masks_np():
    """Diagonal causal mask: [ki, qi] = qi >= ki, duplicated along the free
    axis for the two packed heads."""
    ki = np.arange(128)[:, None]
    qi = np.arange(128)[None, :]
    tri = (qi >= ki).astype(np.float32)
    ones = np.ones((128, 128), np.float32)
    zeros = np.zeros((128, 128), np.float32)
    lo = np.concatenate([tri, ones], axis=1)    # lower k-block of a band
    hi = np.concatenate([zeros, tri], axis=1)   # upper k-block of a band
    return np.concatenate([lo, hi, lo, hi], axis=1).astype(BF16)  # [128, 1024]


def _block(a, w):
    """[C, w] -> [128, NCB*w] partition-blocked bf16."""
    return np.ascontiguousarray(
        a.reshape(NCB, 128, w).transpose(1, 0, 2).reshape(128, NCB * w)).astype(BF16)


def _prep_inputs(x, Wq, Wk, Wv, Wo, t_len):
    masks = _masks_np()
    in_maps = []
    for c in range(N_CORES):
        b, hg = divmod(c, 4)
        sl = slice(hg * SH, (hg + 1) * SH)
        in_maps.append({
            "xT": _block(x[b].T, t_len),
            "wqT": _block(Wq[sl, :].T, SH),
            "wkT": _block(Wk[sl, :].T, SH),
            "wvT": _block(Wv[sl, :].T, SH),
            "woT": _block(Wo[sl, :].T, SH),
            "masks": masks,
        })
    return in_maps


def _assemble(results, t_len):
    out = np.empty((B, t_len, C), dtype=np.float32)
    for c in range(N_CORES):
        b, hg = divmod(c, 4)
        out[b, :, hg * SH:(hg + 1) * SH] = results[c]["out"].T.astype(np.float32)
    return out


def get_nc(t_len=T):
    if t_len not in _CACHE:
        _CACHE[t_len] = _build(t_len)
    return _CACHE[t_len]


def kernel(x, Wq, Wk, Wv, Wo):
    from concourse import bass_utils
    x = np.asarray(x, dtype=np.float32)
    nc = get_nc(T)
    in_maps = _prep_inputs(x, np.asarray(Wq), np.asarray(Wk), np.asarray(Wv),
                           np.asarray(Wo), T)
    res = bass_utils.run_bass_kernel_spmd(nc, in_maps, core_ids=list(range(N_CORES)))
    return _assemble(res.results, T)


# revision 24
# speedup vs baseline: 1.5649x; 1.5322x over previous
"""Distributed causal self-attention kernel for one TRN2 chip (8 NeuronCores).

Problem: y = CausalSelfAttention(x) with B=2, T=2048, C=1024, 16 heads x 64.

Sharding (per core c = b*4 + hg;  b = batch, hg = head-group of 4 heads):
  - Q/K/V projections: column-sharded per head group (each core computes its
    4 heads' Q,K,V from the full x of its batch).
  - Attention: fully local (4 heads per core), flash-style, scores kept
    transposed (s^T[k, q]) so no on-chip transposes are needed.
  - Row-sums for softmax ride the AV matmul as a 65th "ones" column of V.
  - y^T shards are AllGathered within each batch group of 4 cores (two
    gathers, one per head-pair, so comm overlaps the second pair's compute).
  - o_proj: each core computes its own 256 output columns from the full
    gathered y^T -> output shards are disjoint; the host just concatenates.

All matmuls run in bf16 (fp32 accumulation in PSUM); inputs are converted to
bf16 on the host. QK^T matmuls (contraction dim 64) are packed two-per-PE
via tile_position row tiling.
"""
import sys
sys.path.insert(0, '/opt/trn_rl_repo')
import numpy as np
import ml_dtypes

B, T, C = 2, 2048, 1024
NH, HD = 16, 64
N_CORES = 8
GROUPS = [[0, 1, 2, 3], [4, 5, 6, 7]]
HPC = NH // 4            # heads per core = 4
SH = HPC * HD            # per-core projection width = 256
NCB = C // 128           # contraction blocks = 8
QT = 512                 # query tile
BF16 = ml_dtypes.bfloat16

_CACHE = {}


def _build(t_len):
    import concourse.bass as bass
    import concourse.bacc as bacc
    import concourse.tile as tile
    import concourse.mybir as mybir
    dt = mybir.dt
    f32, bf16 = dt.float32, dt.bfloat16

    nqt = t_len // QT        # query tiles
    ntc = t_len // 128       # t chunks of 128
    VW = HPC * 65            # vhat row width = 260

    nc = bacc.Bacc("TRN2", target_bir_lowering=False, debug=False,
                   num_devices=N_CORES)
    # inputs arrive pre-blocked on the host: [(cblk p) ...] -> [p, cblk*...]
    xT = nc.dram_tensor("xT", [128, NCB * t_len], bf16, kind="ExternalInput")
    wq = nc.dram_tensor("wqT", [128, NCB * SH], bf16, kind="ExternalInput")
    wk = nc.dram_tensor("wkT", [128, NCB * SH], bf16, kind="ExternalInput")
    wv = nc.dram_tensor("wvT", [128, NCB * SH], bf16, kind="ExternalInput")
    wo = nc.dram_tensor("woT", [128, NCB * SH], bf16, kind="ExternalInput")
    masks = nc.dram_tensor("masks", [128, 1024], bf16, kind="ExternalInput")
    out = nc.dram_tensor("out", [SH, t_len], bf16, kind="ExternalOutput")

    with tile.TileContext(nc) as tc:
        with tc.tile_pool(name="big", bufs=1) as big, \
             tc.tile_pool(name="epool", bufs=4) as epool, \
             tc.tile_pool(name="small", bufs=3) as small, \
             tc.tile_pool(name="ygp", bufs=8) as ygp, \
             tc.tile_pool(name="stp", bufs=3) as stp, \
             tc.tile_pool(name="ps", bufs=4, space="PSUM") as psp, \
             tc.tile_pool(name="dram", bufs=1, space="DRAM") as dram:

            # ---- resident SBUF tensors ----
            xt = big.tile([128, NCB * t_len], bf16)       # x^T, c-blocked
            wq_sb = big.tile([128, NCB * SH], bf16)
            wk_sb = big.tile([128, NCB * SH], bf16)
            wv_sb = big.tile([128, NCB * SH], bf16)
            wo_sb = big.tile([128, NCB * SH], bf16)
            mask_sb = big.tile([128, 1024], bf16)
            qt_sb = big.tile([128, 2 * t_len], bf16)      # Q^T, pair-blocked
            kt_sb = big.tile([128, 2 * t_len], bf16)
            vhat_sb = big.tile([128, ntc * VW], bf16)     # [V_h | 1] per head

            for k in range(NCB):
                for hh in (0, 1):
                    nc.sync.dma_start(
                        xt[:, k * t_len + hh * (t_len // 2): k * t_len + (hh + 1) * (t_len // 2)],
                        xT[:, k * t_len + hh * (t_len // 2): k * t_len + (hh + 1) * (t_len // 2)])
            for w_sb, w_in in ((wq_sb, wq), (wk_sb, wk), (wv_sb, wv), (wo_sb, wo)):
                for k in range(NCB):
                    nc.sync.dma_start(w_sb[:, k * SH:(k + 1) * SH],
                                      w_in[:, k * SH:(k + 1) * SH])
            nc.sync.dma_start(mask_sb[:], masks[:])
            nc.gpsimd.memset(vhat_sb[:], 1.0)

            # ---- DRAM bounce buffers for the AllGathers (pair x t-half) ----
            n_th = max(1, t_len // 1024)
            th_len = t_len // n_th
            agin = [[dram.tile([128, th_len], bf16, name=f"agin{p}{th}")
                     for th in range(n_th)] for p in (0, 1)]
            agout = [[dram.tile([512, th_len], bf16, name=f"agout{p}{th}")
                      for th in range(n_th)] for p in (0, 1)]

            def qk_proj(pair, w_sb, dst_sb):
                """Q^T/K^T for one head pair: dst rows = head dims (2x64)."""
                for n in range(nqt):
                    ps = psp.tile([128, 512], f32, name="ps")
                    for k in range(NCB):
                        nc.tensor.matmul(
                            ps[:],
                            lhsT=w_sb[:, k * SH + pair * 128: k * SH + (pair + 1) * 128],
                            rhs=xt[:, k * t_len + n * QT: k * t_len + n * QT + QT],
                            start=(k == 0), stop=(k == NCB - 1))
                    nc.vector.tensor_copy(
                        dst_sb[:, pair * t_len + n * QT: pair * t_len + n * QT + QT],
                        ps[:])

            def v_proj():
                """V in [t, o] layout, written per head into vhat (col 65 stays 1)."""
                for tch in range(ntc):
                    ps = psp.tile([128, SH], f32, name="ps")
                    for k in range(NCB):
                        nc.tensor.matmul(
                            ps[:],
                            lhsT=xt[:, k * t_len + tch * 128: k * t_len + (tch + 1) * 128],
                            rhs=wv_sb[:, k * SH:(k + 1) * SH],
                            start=(k == 0), stop=(k == NCB - 1))
                    for h in range(HPC):
                        nc.vector.tensor_copy(
                            vhat_sb[:, tch * VW + h * 65: tch * VW + h * 65 + 64],
                            ps[:, h * 64:(h + 1) * 64])

            def attention(pair):
                def qk_mm(dst, kb, qa, w, h01):
                    """s^T block matmul: k-block kb vs q cols [qa, qa+w)."""
                    nc.tensor.matmul(
                        dst,
                        lhsT=kt_sb[h01 * 64:(h01 + 1) * 64,
                                   pair * t_len + kb * 128: pair * t_len + (kb + 1) * 128],
                        rhs=qt_sb[h01 * 64:(h01 + 1) * 64,
                                  pair * t_len + qa: pair * t_len + qa + w],
                        start=True, stop=True,
                        tile_position=(h01 * 64, 0))

                def av_mm(aug, e_slice, kb, h01, ca, w, start, stop):
                    h = pair * 2 + h01
                    return nc.tensor.matmul(
                        aug[0:65, h01 * 512 + ca: h01 * 512 + ca + w],
                        lhsT=vhat_sb[:, kb * VW + h * 65: kb * VW + (h + 1) * 65],
                        rhs=e_slice,
                        start=start, stop=stop,
                        skip_group_check=True)

                for qi in range(nqt):
                    q0 = qi * QT
                    nfull = q0 // 128          # k-blocks fully valid for all 512 q
                    aug = psp.tile([128, 1024], f32, name="ps")
                    for kb in range(nfull):
                        qk = psp.tile([128, 1024], f32, name="ps")
                        for h01 in (0, 1):
                            qk_mm(qk[:, h01 * 512:(h01 + 1) * 512], kb, q0, 512, h01)
                        e = epool.tile([128, 1024], bf16, name="e")
                        nc.scalar.activation(e[:], qk[:],
                                             mybir.ActivationFunctionType.Exp,
                                             scale=1.0 / np.sqrt(HD))
                        for h01 in (0, 1):
                            av_mm(aug, e[:, h01 * 512:(h01 + 1) * 512], kb, h01,
                                  0, 512, start=(kb == 0), stop=False)
                    # mid supertile: blocks nfull, nfull+1 are fully valid for
                    # the upper q-half [q0+256, q0+512). Packed (i, h01) x 256.
                    mid = psp.tile([128, 1024], f32, name="ps")
                    for i in (0, 1):
                        for h01 in (0, 1):
                            qk_mm(mid[:, (h01 * 2 + i) * 256:(h01 * 2 + i + 1) * 256],
                                  nfull + i, q0 + 256, 256, h01)
                    em = epool.tile([128, 1024], bf16, name="e")
                    nc.scalar.activation(em[:], mid[:],
                                         mybir.ActivationFunctionType.Exp,
                                         scale=1.0 / np.sqrt(HD))
                    for i in (0, 1):
                        for h01 in (0, 1):
                            av_mm(aug, em[:, (h01 * 2 + i) * 256:(h01 * 2 + i + 1) * 256],
                                  nfull + i, h01, 256, 256,
                                  start=(nfull == 0 and i == 0), stop=False)
                    # Two diagonal bands: band u covers q-half [q0+u*256, +256)
                    # against k-blocks nfull+2u, nfull+2u+1 with the causal mask.
                    # PSUM accumulation-group discipline: a start=True matmul
                    # into a bank clobbers any OPEN group in that bank, so band1
                    # (which closes the [256,512) group opened by full/mid) must
                    # fully precede band0's start when nfull==0. Band order
                    # (1, 0) plus an explicit dep enforces this.
                    band_last_av = None
                    band0_first_av = None
                    for u in (1, 0):
                        bd = psp.tile([128, 1024], f32, name="ps")
                        for i in (0, 1):
                            for h01 in (0, 1):
                                qk_mm(bd[:, (h01 * 2 + i) * 256:(h01 * 2 + i + 1) * 256],
                                      nfull + 2 * u + i, q0 + u * 256, 256, h01)
                        eb = epool.tile([128, 1024], bf16, name="e")
                        nc.scalar.activation(eb[:], bd[:],
                                             mybir.ActivationFunctionType.Exp,
                                             scale=1.0 / np.sqrt(HD))
                        nc.vector.tensor_mul(eb[:], eb[:], mask_sb[:])
                        for i in (0, 1):
                            for h01 in (0, 1):
                                av = av_mm(aug, eb[:, (h01 * 2 + i) * 256:(h01 * 2 + i + 1) * 256],
                                           nfull + 2 * u + i, h01, u * 256, 256,
                                           start=(nfull == 0 and u == 0 and i == 0),
                                           stop=(i == 1))
                                if u == 1:
                                    band_last_av = av
                                elif band0_first_av is None:
                                    band0_first_av = av
                    if nfull == 0 and band_last_av is not None:
                        tile.add_dep_helper(band0_first_av.ins, band_last_av.ins,
                                            reason="bank group: band0 start after band1 closes")
                    # normalize: y^T_h = aug[0:64] / aug[64]
                    recip = small.tile([1, 1024], bf16, name="recip")
                    with nc.allow_low_precision(reason="softmax denom in bf16 is within tolerance"):
                        nc.vector.reciprocal(recip[:], aug[64:65, 0:1024])
                    bc = small.tile([64, 1024], bf16, name="bc")
                    nc.gpsimd.partition_broadcast(bc[:], recip[:])
                    yt = small.tile([64, 1024], bf16, name="yt")
                    nc.vector.tensor_mul(yt[:], aug[0:64, 0:1024], bc[:])
                    th, tq = divmod(q0, th_len)
                    nc.sync.dma_start(
                        agin[pair][th].rearrange("(h d) t -> d h t", h=2)[:, :, tq:tq + QT],
                        yt.rearrange("d (h t) -> d h t", h=2))

            # ---- schedule: pair0 projections -> attention p0 (ACT-bound)
            #      overlapped with pair1 projections -> attention p1 ----
            qk_proj(0, wq_sb, qt_sb)
            qk_proj(0, wk_sb, kt_sb)
            v_proj()
            attention(0)
            qk_proj(1, wq_sb, qt_sb)
            qk_proj(1, wk_sb, kt_sb)
            attention(1)

            for th in range(n_th):
                for p in (0, 1):
                    nc.gpsimd.collective_compute(
                        "AllGather", mybir.AluOpType.bypass,
                        replica_groups=GROUPS,
                        ins=[agin[p][th].opt()], outs=[agout[p][th].opt()])

            # ---- gathered y^T -> SBUF ----
            ygt = {}  # (global c-block, t-half) -> sbuf tile
            for th in range(n_th):
                for p in (0, 1):
                    for r in range(4):
                        t = ygp.tile([128, th_len], bf16, name="yg")
                        nc.sync.dma_start(t[:], agout[p][th][r * 128:(r + 1) * 128, :])
                        ygt[(2 * r + p, th)] = t

            def yg(cb, th):
                return ygt[(cb, th)]

            # ---- o_proj: out^T[o, t] = sum_c woT[c, o] * yg[c, t] ----
            nth = th_len // QT
            for n in range(nqt):
                th, nn = divmod(n, nth)
                for m in range(2):
                    ps = psp.tile([128, 512], f32, name="ps")
                    for cb in range(NCB):
                        nc.tensor.matmul(
                            ps[:],
                            lhsT=wo_sb[:, cb * SH + m * 128: cb * SH + (m + 1) * 128],
                            rhs=yg(cb, th)[:, nn * QT: nn * QT + QT],
                            start=(cb == 0), stop=(cb == NCB - 1))
                    st = stp.tile([128, 512], bf16, name="st")
                    nc.vector.tensor_copy(st[:], ps[:])
                    nc.sync.dma_start(
                        out[m * 128:(m + 1) * 128, n * QT: n * QT + QT], st[:])

    nc.compile()
    return nc


def _masks_np():
    """Diagonal causal mask: [ki, qi] = qi >= ki, duplicated along the free
    axis for the two packed heads."""
    ki = np.arange(128)[:, None]
    qi = np.arange(128)[None, :]
    tri = (qi >= ki).astype(np.float32)
    ones = np.ones((128, 128), np.float32)
    zeros = np.zeros((128, 128), np.float32)
    lo = np.concatenate([tri, ones], axis=1)    # lower k-block of a band
    hi = np.concatenate([zeros, tri], axis=1)   # upper k-block of a band
    return np.concatenate([lo, hi, lo, hi], axis=1).astype(BF16)  # [128, 1024]


def _block(a, w):
    """[C, w] -> [128, NCB*w] partition-blocked bf16."""
    return np.ascontiguousarray(
        a.reshape(NCB, 128, w).transpose(1, 0, 2).reshape(128, NCB * w)).astype(BF16)


def _prep_inputs(x, Wq, Wk, Wv, Wo, t_len):
    masks = _masks_np()
    in_maps = []
    for c in range(N_CORES):
        b, hg = divmod(c, 4)
        sl = slice(hg * SH, (hg + 1) * SH)
        in_maps.append({
            "xT": _block(x[b].T, t_len),
            "wqT": _block(Wq[sl, :].T, SH),
            "wkT": _block(Wk[sl, :].T, SH),
            "wvT": _block(Wv[sl, :].T, SH),
            "woT": _block(Wo[sl, :].T, SH),
            "masks": masks,
        })
    return in_maps


def _assemble(results, t_len):
    out = np.empty((B, t_len, C), dtype=np.float32)
    for c in range(N_CORES):
        b, hg = divmod(c, 4)
        out[b, :, hg * SH:(hg + 1) * SH] = results[c]["out"].T.astype(np.float32)
    return out


def get_nc(t_len=T):
    if t_len not in _CACHE:
        _CACHE[t_len] = _build(t_len)
    return _CACHE[t_len]


def kernel(x, Wq, Wk, Wv, Wo):
    from concourse import bass_utils
    x = np.asarray(x, dtype=np.float32)
    nc = get_nc(T)
    in_maps = _prep_inputs(x, np.asarray(Wq), np.asarray(Wk), np.asarray(Wv),
                           np.asarray(Wo), T)
    res = bass_utils.run_bass_kernel_spmd(nc, in_maps, core_ids=list(range(N_CORES)))
    return _assemble(res.results, T)
